# revision 6
# baseline (speedup 1.0000x reference)
"""Trainium2 kernel for nn_CIE_18236431138961 (embedding_lookup family).

Reference computation (per batch n, feature d):
    idx   = argsort-descending of x[n, :, d]            (S=16 sources)
    gaps  = consecutive differences of sorted values (last gap = last value)
    codes = cumulative bitmask of the top-k index set at each sort position
    table[c] = sum_j FM[source_index[c, j]] * Agg[0, j]  (c in [0, 2^S-1))
    out[n, :, d] = sum_s gaps[s] * table[codes[s]]       (a Choquet integral)

Key identity: the shipped source_index encodes row c as the bit pattern of
c+1, so table is ADDITIVE over bits:  table[c] = C + sum_{j in bits(c+1)} V[j]
with V[j] = table[{j}] - C and C = table[{0}]+table[{1}]-table[{0,1}].
For an additive (set-function) table the Choquet integral telescopes:
    sum_s gaps[s] * table[codes[s]]
      = sum_t x_sort[t] * V[idx[t]] + C * sum_s gaps[s]
      = sum_j x[n, j, d] * V[j]     + C * max_s x[n, s, d]
(the first term because idx is a permutation, the second because the gap sum
telescopes to the max).  With the reference FM (row 0 is the zero row) C == 0
exactly, and the whole pipeline is a single tiny contraction:
    out[n, h, d] = sum_s x[n, s, d] * V[s, h]

kernel() verifies this structure numerically on the host from the actual
inputs (so correctness never depends on the assumption), then runs the
contraction on 8 NeuronCores, data-parallel over the batch axis. If the
structure check ever fails (non-additive table), it falls back to a faithful
numpy implementation of the reference math.
"""

import numpy as np

N, S, D, H = 128, 16, 512, 4
NCORES = 8
NPC = N // NCORES          # batch rows per core
GROUPS = NPC // 8          # 8 batch rows per matmul (8*16 sources = 128 = K)

_BASS_CACHE = {}

# test.py hooks (harness never touches these)
TRACE = False
TRACE_KWARGS = {}
LAST_RESULTS = None


def _build_affine_nc():
    """Bass program (one NeuronCore, SPMD x8): out = blockdiag(V).T @ x.

    Inputs (per core):
      xs  [128, NPC*64] f32 : x shard rearranged so partition p = 16*j + s
                              (j = batch-in-group, s = source), free = (g, d)
      w   [128, 32] f32     : block-diagonal weights, w[16j+s, 4j+h] = V[s, h]
    Output:
      out [NPC*4, 512] f32  : rows g*32 + 4j + h  ->  out[8g+j, h, :]
    """
    import concourse.bass as bass
    import concourse.mybir as mybir
    from contextlib import ExitStack

    f32 = mybir.dt.float32
    XCOLS = GROUPS * 512
    nc = bass.Bass()
    xw = nc.dram_tensor("xw", [128, XCOLS + 32], f32, kind="ExternalInput")
    out = nc.dram_tensor("out", [GROUPS * 32, 512], f32, kind="ExternalOutput")

    with ExitStack() as ctx:
        xt = ctx.enter_context(nc.sbuf_tensor([128, XCOLS + 32], f32))
        ot = ctx.enter_context(nc.sbuf_tensor([32, GROUPS * 512], f32))
        pts = [
            ctx.enter_context(nc.psum_tensor(f"pt{g}", [32, 512], f32))
            for g in range(GROUPS)
        ]
        in_sem = ctx.enter_context(nc.semaphore())
        mm_sem = ctx.enter_context(nc.semaphore())
        cp_sem = ctx.enter_context(nc.semaphore())
        out_sem = ctx.enter_context(nc.semaphore())
        block = ctx.enter_context(nc.Block())

        @block.tensor
        def _(tensor):
            tensor.wait_ge(in_sem, 16)
            for g in range(GROUPS):
                nc.tensor.matmul(
                    out=pts[g][:],
                    lhsT=xt[:, XCOLS:XCOLS + 32],
                    rhs=xt[:, g * 512:(g + 1) * 512],
                    start=True,
                    stop=True,
                ).then_inc(mm_sem, 1)

        @block.vector
        def _(vector):
            for g in range(GROUPS):
                vector.wait_ge(mm_sem, g + 1)
                nc.vector.tensor_copy(
                    out=ot[:, g * 512:(g + 1) * 512], in_=pts[g][:]
                ).then_inc(cp_sem, 1)

        @block.sync
        def _(sync):
            sync.dma_start(out=xt[:], in_=xw[:]).then_inc(in_sem, 16)
            for g in range(GROUPS):
                sync.wait_ge(cp_sem, g + 1)
                sync.dma_start(
                    out=out[g * 32:(g + 1) * 32, :],
                    in_=ot[:, g * 512:(g + 1) * 512],
                ).then_inc(out_sem, 16)
            sync.wait_ge(out_sem, GROUPS * 16)
    return nc


def _run_affine(x, V):
    """x (N,S,D) f32, V (S,H) f32 -> out (N,H,D) f32 via 8-core SPMD matmul."""
    global LAST_RESULTS
    from concourse.bass_utils import run_bass_kernel_spmd

    if "affine" not in _BASS_CACHE:
        _BASS_CACHE["affine"] = _build_affine_nc()
    nc = _BASS_CACHE["affine"]

    # block-diagonal lhsT: rows 16j+s, cols 4j+h
    w = np.zeros((128, 32), np.float32)
    for j in range(8):
        w[16 * j:16 * (j + 1), 4 * j:4 * (j + 1)] = V

    core_ids = list(range(NCORES))
    in_maps = []
    for c in core_ids:
        shard = x[c * NPC:(c + 1) * NPC]                  # (NPC, S, D)
        xs = shard.reshape(GROUPS, 128, 512).transpose(1, 0, 2).reshape(128, -1)
        in_maps.append({"xw": np.ascontiguousarray(np.concatenate([xs, w], axis=1))})

    res = run_bass_kernel_spmd(
        nc, in_maps, core_ids, trace=TRACE, **TRACE_KWARGS
    )
    LAST_RESULTS = res
    out = np.empty((N, H, D), np.float32)
    for c in core_ids:
        out[c * NPC:(c + 1) * NPC] = res.results[c]["out"].reshape(NPC, H, D)
    return out


def _general_fallback(x, table):
    """Faithful numpy mirror of the reference for non-additive tables."""
    idx = np.argsort(-x, axis=1, kind="stable")
    x_sort = np.take_along_axis(x, idx, axis=1)
    gaps = np.concatenate(
        [x_sort[:, :-1] - x_sort[:, 1:], x_sort[:, -1:]], axis=1
    )
    codes = np.cumsum((1 << idx.astype(np.int64)).astype(np.int32), axis=1) - 1
    fm = table[codes]                                     # (N,S,D,H)
    out = np.einsum("nsd,nsdh->ndh", gaps, fm)
    return np.ascontiguousarray(out.transpose(0, 2, 1).astype(np.float32))


def kernel(**inputs):
    x = np.ascontiguousarray(np.asarray(inputs["x"], dtype=np.float32))
    FM = np.asarray(inputs["FM"], dtype=np.float32)
    Agg = np.asarray(inputs["Agg"], dtype=np.float32)
    si = np.asarray(inputs["source_index"])

    # Host-side param preprocessing: per-code reduction table (65535, H).
    table = (FM[si] * Agg[0][None, :, :]).sum(1).astype(np.float32)

    # Affine fit over the bit pattern of c+1.
    C = table[0] + table[1] - table[2]                    # {0}+{1}-{0,1}
    V = table[(1 << np.arange(S)) - 1] - C                # (S, H) singletons
    bits = ((np.arange(1, 2 ** S)[:, None] >> np.arange(S)) & 1).astype(
        np.float32
    )
    recon = C[None, :] + bits @ V
    scale = max(float(np.abs(table).max()), 1e-12)
    affine = float(np.abs(recon - table).max()) <= 1e-4 * scale
    c_zero = float(np.abs(C).max()) <= 1e-5 * scale

    if affine and c_zero:
        return _run_affine(x, V.astype(np.float32))
    return _general_fallback(x, table)
